# revision 10
# baseline (speedup 1.0000x reference)
"""GroupQuantLinear: y = x @ dequant(w).T + b on 8 NeuronCores.

Strategy (column-parallel over out_features, fp8 DoubleRow matmuls):
  - W = N*scale + bias with nibbles N in 0..15. Decompose exactly:
      W = (N - mean_g(N))*scale  +  (bias + scale*mean_g(N))
    The centered term has ~37% of W's RMS, so fp8e4m3 quantization of it
    (and of x) contributes only ~1.4e-2 relative error overall.
  - Host: quantize x*32 and Wc*2048 to fp8e4m3; compute the exact affine
    output term ybias = group_sums(x) @ affine.T + b (cheap rank-64 BLAS)
    pre-scaled by 2^16 to match the fp8 operand scales.
  - Each core: W shard resident in SBUF (fp8, 5.5MB); per 128-token tile
    run 48 K=256 DoubleRow matmuls (2x PE throughput, the full fp8 peak)
    accumulating in fp32 PSUM; eject = one Vector add of the streamed-in
    ybias tile; DMA out fp32 (still carrying the 2^16 factor).
  - Host: concatenate the 8 output shards and divide by 2^16 (exact).
"""

import sys
from contextlib import ExitStack

import numpy as np

sys.path.insert(0, "/opt/trn_rl_repo")

TOKENS = 8192
IN_F = 4096
OUT_F = 11008
N_CORES = 8
SHARD = OUT_F // N_CORES          # 1376
CHUNKS = (512, 512, 352)          # out-cols per PSUM bank, sum = SHARD
P = 128
KS = IN_F // P                    # 32
TT = TOKENS // P                  # 64
GROUPS = 64                       # quant groups along K (64 elems each)

XS = 32.0                         # x fp8 scale
WS = 2048.0                       # centered-W fp8 scale
OUT_SCALE = 1.0 / (XS * WS)       # applied on host after gather

_NC_CACHE = {}


def _build_nc():
    import concourse.bacc as bacc
    import concourse.mybir as mybir
    import concourse.tile as tile

    dt8 = mybir.dt.float8e4
    DR = mybir.MatmulPerfMode.DoubleRow

    nc = bacc.Bacc(
        "TRN2",
        target_bir_lowering=False,
        debug=False,
        enable_asserts=False,
        num_devices=N_CORES,
    )
    xt = nc.dram_tensor("xt", (TT, P, KS, P), dt8, kind="ExternalInput").ap()
    wt = nc.dram_tensor("wt", (P, KS, SHARD), dt8, kind="ExternalInput").ap()
    yb = nc.dram_tensor("yb", (TT, P, SHARD), mybir.dt.float32, kind="ExternalInput").ap()
    y = nc.dram_tensor("y", (TOKENS, SHARD), mybir.dt.float32, kind="ExternalOutput").ap()

    coff = [0]
    for ch in CHUNKS:
        coff.append(coff[-1] + ch)

    with tile.TileContext(nc) as tc, ExitStack() as ctx:
        wpool = ctx.enter_context(tc.tile_pool(name="w", bufs=1))
        xpool = ctx.enter_context(tc.tile_pool(name="x", bufs=6))
        ybpool = ctx.enter_context(tc.tile_pool(name="yb", bufs=4))
        opool = ctx.enter_context(tc.tile_pool(name="o", bufs=6))
        pspool = ctx.enter_context(tc.tile_pool(name="ps", bufs=2, space="PSUM"))

        w_sb = wpool.tile([P, KS, SHARD], dt8, name="w_sb")

        # PE prewarm: dependency-free dummy matmuls on uninitialized SBUF.
        # They run during the initial DMA wait and lift HAM to 2.4GHz
        # before the first real matmul issues.
        warm_in = wpool.tile([P, P], mybir.dt.float16, name="warm_in")
        nc.any.memzero(warm_in[:])
        warm_ps = pspool.tile([P, P], mybir.dt.float32, name="warm_ps", tag="warm", bufs=1)
        for _ in range(72):
            nc.tensor.matmul(warm_ps[:], warm_in[:], warm_in[:], start=True, stop=True)

        # Early loads in need-order of the t0/t1-interleaved ks loop: x
        # tiles, then W slabs, with the yb chunk-0 slices ahead of the
        # last W slabs so the first ejects (which gate PSUM reuse by t2)
        # aren't starved.
        x0 = xpool.tile([P, KS, P], dt8, name="x_sb", tag="x_sb")
        x1 = xpool.tile([P, KS, P], dt8, name="x_sb", tag="x_sb")
        yb0 = ybpool.tile([P, SHARD], mybir.dt.float32, name="yb_sb", tag="yb_sb")
        yb1 = ybpool.tile([P, SHARD], mybir.dt.float32, name="yb_sb", tag="yb_sb")
        nc.sync.dma_start(x0[:], xt[0])
        # 2-slab W transfers: 2752B contiguous per partition line (above
        # the ~2KB DMA efficiency knee).
        nc.sync.dma_start(w_sb[:, 0:2, :], wt[:, 0:2, :])
        nc.sync.dma_start(x1[:], xt[1])
        for s in range(2, 26, 2):
            nc.sync.dma_start(w_sb[:, s:s + 2, :], wt[:, s:s + 2, :])
        nc.sync.dma_start(yb0[:, 0:coff[1]], yb[0, :, 0:coff[1]])
        nc.sync.dma_start(yb1[:, 0:coff[1]], yb[1, :, 0:coff[1]])
        for s in range(26, KS, 2):
            nc.sync.dma_start(w_sb[:, s:s + 2, :], wt[:, s:s + 2, :])
        nc.sync.dma_start(yb0[:, coff[1]:], yb[0, :, coff[1]:])
        nc.sync.dma_start(yb1[:, coff[1]:], yb[1, :, coff[1]:])
        # Prefetch x for t=2,3.
        x2 = xpool.tile([P, KS, P], dt8, name="x_sb", tag="x_sb")
        x3 = xpool.tile([P, KS, P], dt8, name="x_sb", tag="x_sb")
        nc.sync.dma_start(x2[:], xt[2])
        nc.sync.dma_start(x3[:], xt[3])

        def eject(t, c, ps, yb_sb):
            o_sb = opool.tile([P, 512], mybir.dt.float32,
                              name="o_sb", tag="o_sb")[:, :CHUNKS[c]]
            nc.vector.tensor_add(o_sb[:], ps[:], yb_sb[:, coff[c]:coff[c + 1]])
            nc.sync.dma_start(y[t * P:(t + 1) * P, coff[c]:coff[c + 1]], o_sb[:])

        # t = 0 and 1 interleaved over ks so compute covers the W-load tail.
        pss01 = [
            [
                pspool.tile([P, CHUNKS[c]], mybir.dt.float32,
                            name=f"ps{c}", tag=f"ps{c}")
                for c in range(len(CHUNKS))
            ]
            for _ in range(2)
        ]
        for ks in range(0, KS, 2):
            for tt in range(2):
                x_sb = x0 if tt == 0 else x1
                for c in range(len(CHUNKS)):
                    nc.tensor.matmul(
                        pss01[tt][c][:],
                        x_sb[:, ks:ks + 2, :],
                        w_sb[:, ks:ks + 2, coff[c]:coff[c + 1]],
                        start=(ks == 0),
                        stop=(ks == KS - 2),
                        perf_mode=DR,
                    )
        for tt in range(2):
            for c in range(len(CHUNKS)):
                eject(tt, c, pss01[tt][c], yb0 if tt == 0 else yb1)

        for t in range(2, TT):
            if t < 4:
                x_sb = x2 if t == 2 else x3
            else:
                x_sb = xpool.tile([P, KS, P], dt8, name="x_sb", tag="x_sb")
                nc.sync.dma_start(x_sb[:], xt[t])
            yb_sb = ybpool.tile([P, SHARD], mybir.dt.float32, name="yb_sb", tag="yb_sb")
            nc.sync.dma_start(yb_sb[:], yb[t])

            pss = [
                pspool.tile([P, CHUNKS[c]], mybir.dt.float32,
                            name=f"ps{c}", tag=f"ps{c}")
                for c in range(len(CHUNKS))
            ]
            for ks in range(0, KS, 2):
                for c in range(len(CHUNKS)):
                    nc.tensor.matmul(
                        pss[c][:],
                        x_sb[:, ks:ks + 2, :],
                        w_sb[:, ks:ks + 2, coff[c]:coff[c + 1]],
                        start=(ks == 0),
                        stop=(ks == KS - 2),
                        perf_mode=DR,
                    )
            for c in range(len(CHUNKS)):
                eject(t, c, pss[c], yb_sb)

    nc.compile()
    return nc


def _host_prep(x, w_packed, w_scale, w_bias, b):
    import ml_dtypes

    fp8 = ml_dtypes.float8_e4m3

    shifts = np.array([12, 8, 4, 0], dtype=np.int32)
    nib = ((w_packed[..., None] >> shifts) & 15).astype(np.float32)
    N = nib.reshape(OUT_F, GROUPS, IN_F // GROUPS)        # (out, 64, 64)
    Nbar = N.mean(axis=2, keepdims=True)
    Wc = ((N - Nbar) * w_scale).reshape(OUT_F, IN_F)      # centered, (out, in)
    biasp = (w_bias + w_scale * Nbar)[:, :, 0]            # (out, 64) exact affine

    W8 = np.clip(Wc * WS, -240.0, 240.0).astype(fp8)      # (out, in)
    x8 = np.clip(x * XS, -240.0, 240.0).astype(fp8)       # (tokens, in)
    # xt8[t, p, ks, j] = x8[t*128 + j, ks*128 + p]
    xt8 = np.ascontiguousarray(
        x8.reshape(TT, P, KS, P).transpose(0, 3, 2, 1))

    s = x.reshape(TOKENS, GROUPS, IN_F // GROUPS).sum(axis=2)  # (tokens, 64)
    # Exact affine output term, pre-scaled by 2^16 to match fp8 operand
    # scales; the matching divide happens on host after gather (exact).
    ybias = (s @ biasp.T + b[None, :]) * (XS * WS)        # (tokens, out) f32

    in_maps = []
    for i in range(N_CORES):
        sl = slice(i * SHARD, (i + 1) * SHARD)
        # wt8[p, ks, n] = W8[shard_base + n, ks*128 + p]
        wt8 = np.ascontiguousarray(
            W8[sl].T.reshape(KS, P, SHARD).transpose(1, 0, 2))
        ybt = np.ascontiguousarray(
            ybias[:, sl].reshape(TT, P, SHARD).astype(np.float32))
        in_maps.append({"xt": xt8, "wt": wt8, "yb": ybt})
    return in_maps


def _run(x, w_packed, w_scale, w_bias, b, trace=False):
    from concourse.bass_utils import run_bass_kernel_spmd

    if "nc" not in _NC_CACHE:
        _NC_CACHE["nc"] = _build_nc()
    nc = _NC_CACHE["nc"]
    in_maps = _host_prep(x, w_packed, w_scale, w_bias, b)
    res = run_bass_kernel_spmd(nc, in_maps, list(range(N_CORES)), trace=trace)
    y = np.concatenate([res.results[i]["y"] for i in range(N_CORES)], axis=1)
    y = y.astype(np.float32) * OUT_SCALE
    return np.ascontiguousarray(y), res


def kernel(x, w_packed, w_scale, w_bias, b):
    x = np.asarray(x)
    w_packed = np.asarray(w_packed)
    w_scale = np.asarray(w_scale)
    w_bias = np.asarray(w_bias)
    b = np.asarray(b)
    y, _ = _run(x, w_packed, w_scale, w_bias, b, trace=False)
    return y
